# revision 43
# baseline (speedup 1.0000x reference)
"""FPN ROI-Align pooler (nn_Pooler) as a Bass/Tile kernel on 8 Trainium2 cores.

Design (v4):
  - Host builds a 2-row-banded channels-last table: entry (img,y,x) holds
    feature rows y and y+1 for pixel column x -> [106250+pad, 2*256], in both
    fp16 and fp8-e4m3. Byte-heavy bin classes (far-sample bins plus half of
    the (3,2) class, ~37% of bins) gather from the fp8 copy, halving their
    HBM traffic for ~1.6e-2 rel err (gate 2e-2).
  - Each output bin needs noff fetches of a K-col x 2-row window: per (bin,
    offset) one gathered chunk of K consecutive band entries (K*2*256 contig).
    Partition = (bin, y-half): 128 chunk-slots per tile of ts bins.
  - Gathers run through InstDMAGatherAnt (dma_gather): int16 indices relative
    to overlapping 32768-row table windows (base stride 26563 rows), up to 8
    tiles (1024 indices, the device SWDGE descriptor-ring cap) per gather
    instruction -> SWDGE fixed cost amortized. Trailing pad slots of each
    stream's final chunk are skipped by shrinking num_idxs; the ragged tile's
    matmuls contract only the written partitions.
  - Bilinear + sample-average reduce to 2K weighted terms per slot, done
    on PE as 2K block-diagonal matmuls accumulating in PSUM [ts,256] f32.
    DVE builds the block-diag weight tiles (broadcast x mask), ACT packs
    PSUM->SBUF output slabs, one merged store DMA per chunk.
  - Chunk emission interleaves the (K, dtype) gather streams round-robin
    (fp8 first: 2x PE work per DMA byte) so gathers/compute overlap and the
    drain tail stays DMA-matched.
  - Bins are classified by x-window K in {2,3,4} and noff in {1,2,4}, dealt
    bin-level round-robin to the 8 cores so every core runs the identical
    SPMD program on balanced work.
"""

import sys

import numpy as np

if "/opt/trn_rl_repo" not in sys.path:
    sys.path.insert(0, "/opt/trn_rl_repo")

OUT = 7
SR = 2
SCALES = (0.25, 0.125, 0.0625, 0.03125)
K_MIN = 2
CANON_SCALE = 224.0
CANON_LVL = 4.0
EPS = 1e-6

B, C, N = 2, 256, 1000
SIZES = ((200, 200), (100, 100), (50, 50), (25, 25))
NCORES = 8
NBIN = OUT * OUT
# class config: (K, noff) = x-window width in band entries, offsets per bin.
# Classes 7/8 are the far-sample classes, gathered from an fp8-e4m3 copy of
# the table (they carry 34% of the bytes; quantization adds ~1e-2 rel err).
CFG = ((2, 1), (2, 2), (2, 4), (3, 1), (3, 2), (4, 1), (4, 2),
       (2, 2), (2, 4), (3, 2))
FP8_CLS = frozenset((7, 8, 9))
FP8_32_FRAC = 1, 2  # route (3,2) bins with flat_idx % 2 < 1 through fp8
CLS_BY = {cfg: i for i, cfg in enumerate(CFG[:7])}
MAXNOFF = 4
MAXNT = 8  # max terms per offset (2K with K<=4)

LEVEL_BASE = []
_acc = 0
for _h, _w in SIZES:
    LEVEL_BASE.append(_acc)
    _acc += B * _h * _w
TOTAL_ROWS = _acc  # 106250
PAD_ROWS = 32
TBL_ROWS = TOTAL_ROWS + PAD_ROWS

WBASE = 26563          # window base stride (rows); 4 windows cover the table
WSPAN = 32768          # int16-addressable window span (rows)
NWINDOWS = (TOTAL_ROWS - 1) // WBASE + 1  # = 4
# gather chunk size (tiles per dma_gather): capped at 1024 indices per
# instruction (SWDGE descriptor-ring limit on the device)
CHUNK_TILES = {2: 8, 3: 6, 4: 4}
CHUNK_TILES_F8 = {2: 8, 3: 6}  # fp8 g tile is 18432 elems (1B each)

_PROGRAM_CACHE: dict = {}


def _axis_precompute(lo, hi, Wdim):
    """Per-axis samples: corners [N,14,2] i32, weights [N,14,2] f32, valid."""
    f32 = np.float32
    roi = np.maximum(hi - lo, f32(1.0))
    bin_sz = roi / f32(OUT)
    a_out = np.arange(OUT, dtype=f32)[None, :, None]
    a_sr = np.arange(SR, dtype=f32)[None, None, :]
    grid = a_out * bin_sz[:, None, None] + (a_sr + f32(0.5)) * bin_sz[:, None, None] / f32(SR)
    pos = (lo[:, None, None] + grid).reshape(N, OUT * SR)
    Wf = Wdim.astype(f32)
    valid = (pos >= f32(-1.0)) & (pos <= Wf[:, None])
    p = np.clip(pos, f32(0.0), (Wf - f32(1.0))[:, None])
    p0f = np.floor(p)
    p0 = p0f.astype(np.int32)
    p1 = np.minimum(p0 + 1, Wdim[:, None] - 1)
    lp = p - p0f
    hp = f32(1.0) - lp
    corn = np.stack([p0, p1], axis=-1)
    wgt = np.stack([hp, lp], axis=-1).astype(f32)
    return corn, wgt, valid


def _host_precompute(boxes, img_ids):
    """Returns cls [Nb], idxP [Nb,4] i32 band-entry offsets, wc [Nb,4,8] f32
    (j = d*2+r), using banded-table rows."""
    f32 = np.float32
    boxes = np.asarray(boxes, f32)
    x1, y1, x2, y2 = boxes[:, 0], boxes[:, 1], boxes[:, 2], boxes[:, 3]
    area = (x2 - x1) * (y2 - y1)
    s = np.sqrt(area)
    lvl = np.floor(f32(CANON_LVL) + np.log2(s / f32(CANON_SCALE) + f32(EPS)))
    lvl = np.clip(lvl, K_MIN, K_MIN + len(SCALES) - 1).astype(np.int32) - K_MIN

    scale = np.asarray(SCALES, f32)[lvl]
    Hs = np.asarray([h for h, w in SIZES], np.int32)[lvl]
    Ws = np.asarray([w for h, w in SIZES], np.int32)[lvl]
    base = np.asarray(LEVEL_BASE, np.int64)[lvl]

    xcorn, xw, xval = _axis_precompute(x1 * scale, x2 * scale, Ws)
    ycorn, yw, yval = _axis_precompute(y1 * scale, y2 * scale, Hs)

    # --- x side: per (n, bx): window start, per-pixel weights, class
    ix = (np.arange(OUT)[:, None] * SR + np.arange(SR)[None, :])  # [7,2]
    x0a = xcorn[:, ix[:, 0], 0]  # [N,7] window start (first sample lo corner)
    dcorn = xcorn[:, ix, :] - x0a[:, :, None, None]  # [N,7,2kx,2cx]
    assert dcorn.min() >= 0
    dmax = dcorn.max(axis=(2, 3))  # [N,7]
    assert dmax.max() <= 15, f"x window overflow: {dmax.max()}"
    kclass = np.zeros((N, OUT), np.int32)  # K=2
    kclass[dmax > 1] = 1  # K=3
    kclass[dmax > 2] = 2  # K=4
    kclass[dmax > 3] = 3  # far (per-sample 2-col fetches)

    wx_pix = np.zeros((N, OUT, 16), f32)
    wxc = (xw[:, ix, :] * xval[:, ix][:, :, :, None]) * f32(0.5)  # [N,7,2,2]
    n_i, b_i = np.meshgrid(np.arange(N), np.arange(OUT), indexing="ij")
    for kx in range(SR):
        for cx in range(2):
            np.add.at(wx_pix, (n_i, b_i, dcorn[:, :, kx, cx]), wxc[:, :, kx, cx])

    # --- y side: per (n, by, s): band entry row and 2 row-weights
    iy = (np.arange(OUT)[:, None] * SR + np.arange(SR)[None, :])  # [7,2]
    ybase = ycorn[:, iy, 0]  # [N,7,2]
    wyr = (yw[:, iy, :] * yval[:, iy][:, :, :, None]) * f32(0.5)  # [N,7,2s,2r]

    img = np.asarray(img_ids).astype(np.int64)
    rowy = (img[:, None, None] * Hs[:, None, None].astype(np.int64)
            + ybase.astype(np.int64))  # [N,7,2]
    idx = (base[:, None, None, None]
           + rowy[:, :, None, :] * Ws[:, None, None, None].astype(np.int64)
           + x0a.astype(np.int64)[:, None, :, None])  # [N,by,bx,s]
    assert idx.min() >= 0 and idx.max() < TOTAL_ROWS
    # far bins: per-x-sample window starts
    x0s = xcorn[:, ix, 0]  # [N,7,2xs] per-sample lo corner
    idx4 = (base[:, None, None, None, None]
            + rowy[:, :, None, :, None] * Ws[:, None, None, None, None].astype(np.int64)
            + x0s.astype(np.int64)[:, None, :, None, :])  # [N,by,bx,s,xs]

    ym = idx[..., 0] == idx[..., 1]  # [N,7,7]
    Nb = N * NBIN
    cls = np.zeros((N, OUT, OUT), np.int32)
    idxP = np.zeros((N, OUT, OUT, MAXNOFF), np.int64)
    wcP = np.zeros((N, OUT, OUT, MAXNOFF, MAXNT), f32)

    wc = (wyr[:, :, None, :, None, :] * wx_pix[:, None, :, None, :, None])
    # wc: [n, by, bx, s, d, r]
    wc_m = wc.sum(axis=3)  # [n,by,bx,d,r] y-merged
    kc3 = np.repeat(kclass[:, None, :], OUT, axis=1)  # [n,by,bx] 0..3
    for ki, Kv in ((0, 2), (1, 3), (2, 4)):
        m1 = (kc3 == ki) & ym
        m2 = (kc3 == ki) & ~ym
        cls[m1] = CLS_BY[(Kv, 1)]
        cls[m2] = CLS_BY[(Kv, 2)]
        nT = 2 * Kv
        wnear = wc[..., :Kv, :].reshape(N, OUT, OUT, 2, nT)
        wmerg = wc_m[..., :Kv, :].reshape(N, OUT, OUT, nT)
        idxP[m2, :2] = idx[m2]
        wcP[m2, 0, :nT] = wnear[m2][:, 0]
        wcP[m2, 1, :nT] = wnear[m2][:, 1]
        idxP[m1, 0] = idx[m1][:, 0]
        wcP[m1, 0, :nT] = wmerg[m1]
    # far classes: per (s, xs) 2-col fetches, terms j = d*2+r, d in {0,1}
    wxh = wxc  # [N,7(bx),2xs,2cx] per-sample x weights (incl valid*0.5)
    wfar = (wyr[:, :, None, :, None, None, :]
            * wxh[:, None, :, None, :, :, None])  # [n,by,bx,s,xs,d,r]
    wfar = wfar.reshape(N, OUT, OUT, 2, 2, 4)  # [.., s, xs, j]
    mfar = kc3 == 3
    mf2 = mfar & ym   # far-ym: noff=2, one fetch per xs
    mf4 = mfar & ~ym  # far-split: noff=4, fetch per (s, xs)
    cls[mf4] = 8
    cls[mf2] = 7
    # a slice of the byte-heavy (3,2) class also goes fp8: the quantization
    # error budget allows ~1/3 of all bins at fp8 under the 2e-2 gate
    flat = np.arange(Nb).reshape(N, OUT, OUT)
    sel = (cls == CLS_BY[(3, 2)]) & (flat % FP8_32_FRAC[1] < FP8_32_FRAC[0])
    cls[sel] = 9
    idxP[mf4] = idx4[mf4].reshape(-1, 4)
    wcP[mf4, :, :4] = wfar[mf4].reshape(-1, 4, 4)
    idxP[mf2, :2] = idx4[mf2][:, 0]  # rows equal for both s
    wcP[mf2, :2, :4] = wfar[mf2].sum(axis=1)  # sum over s
    return cls.reshape(Nb), idxP.reshape(Nb, MAXNOFF).astype(np.int32), \
        np.ascontiguousarray(wcP.reshape(Nb, MAXNOFF, MAXNT), dtype=f32)


def _make_table(feats, dtype):
    """2-row-banded channels-last table [TBL_ROWS, 2*C]."""
    parts = []
    for f in feats:
        _, _, H, W = f.shape
        nhwc = np.ascontiguousarray(f.transpose(0, 2, 3, 1))  # [B,H,W,C]
        padded = np.concatenate([nhwc, np.zeros((B, 1, W, C), f.dtype)], axis=1)
        band = np.stack([padded[:, :H], padded[:, 1:H + 1]], axis=3)  # [B,H,W,2,C]
        parts.append(band.reshape(-1, 2 * C))
    parts.append(np.zeros((PAD_ROWS, 2 * C), parts[0].dtype))
    return np.ascontiguousarray(np.concatenate(parts, axis=0)).astype(dtype)


def _tile_bins(ci):
    return 128 // CFG[ci][1]


def _bin_windows(cls, idxP):
    """Per-bin window id from the min valid fetch offset."""
    Nb = cls.shape[0]
    noffs = np.array([c[1] for c in CFG])[cls]
    mask = np.arange(MAXNOFF)[None, :] < noffs[:, None]
    big = np.where(mask, idxP, np.iinfo(np.int32).max)
    mn = big.min(axis=1)
    mx = np.where(mask, idxP, -1).max(axis=1)
    w = np.minimum(mn // WBASE, NWINDOWS - 1)
    assert (mn - w * WBASE >= 0).all()
    assert (mx - w * WBASE < WSPAN).all(), "bin fetch spread exceeds window"
    return w.astype(np.int32)


def _plan(cls, idxP):
    """Deal bins to cores (bin-level round-robin) per (class, window).
    Returns plans dict [(ci,w)] -> [NCORES, ni] bin ids and the chunk
    schedule: list of (K, w, tiles=[(ci, slot_lo, ns)], num_idxs). Ragged
    tiles go last in each (K,w) stream so the final chunk's trailing pad
    slots can be skipped by shrinking num_idxs."""
    win = _bin_windows(cls, idxP)
    plans = {}
    for ci in range(len(CFG)):
        for w in range(NWINDOWS):
            ids = np.where((cls == ci) & (win == w))[0]
            if len(ids) == 0:
                continue
            n8 = int(np.ceil(len(ids) / NCORES)) * NCORES
            padded = -np.ones((n8,), np.int64)
            padded[:len(ids)] = ids
            plans[(ci, w)] = padded.reshape(-1, NCORES).T  # [8, ni]
    chunks = []
    streams = [(K, False, [ci for ci in range(7) if CFG[ci][0] == K])
               for K in (2, 3, 4)]
    streams.append((2, True, [7, 8]))
    streams.append((3, True, [9]))
    for K, f8, cls_of_k in streams:
        for w in range(NWINDOWS):
            full, ragged = [], []
            for ci in cls_of_k:
                if (ci, w) not in plans:
                    continue
                noff = CFG[ci][1]
                slots = plans[(ci, w)].shape[1] * noff
                for k in range(slots // 128):
                    full.append((ci, 128 * k, 128))
                if slots % 128:
                    ragged.append((ci, 128 * (slots // 128), slots % 128))
            stream = full + ragged
            m = CHUNK_TILES_F8[K] if f8 else CHUNK_TILES[K]
            for a in range(0, len(stream), m):
                part = stream[a:a + m]
                ns_last = part[-1][2]
                num_idxs = 128 * (len(part) - 1) + ((ns_last + 15) // 16) * 16
                chunks.append((K, w, tuple(part), num_idxs, f8))
    # interleave chunk emission round-robin across stream kinds so adjacent
    # chunks are independent (better gather/compute overlap)
    groups = {}
    for ch in chunks:
        groups.setdefault((ch[0], ch[4]), []).append(ch)
    # fp8 chunks carry 2x the PE work per DMA byte; drawing them first each
    # round front-loads compute so the schedule tail stays DMA-matched
    gl = [g_ for k_, g_ in groups.items() if k_[1]]
    gl += [g_ for k_, g_ in groups.items() if not k_[1]]
    order = []
    while any(gl):
        for g_ in gl:
            if g_:
                order.append(g_.pop(0))
    return plans, order


def _sig(chunks):
    return tuple(chunks)


def _pack_core(core, plans, chunks, idxP, wcP):
    """Per-core DRAM inputs following the chunk schedule:
    idx_arr [128, Stot] i16 (16-part wrap, replicated x8),
    wc_arr [128, wc_cols] f16, slotmap [out_rows] bin ids."""
    Stot = sum(8 * len(tl) for _, _, tl, _, _ in chunks)
    wc_cols = sum(2 * K * len(tl) for K, _, tl, _, _ in chunks)
    out_rows = sum(_tile_bins(ci) for _, _, tl, _, _ in chunks for ci, _, _ in tl)
    idx_arr = np.zeros((128, Stot), np.int16)
    wc_arr = np.zeros((128, wc_cols), np.float16)
    slotmap = np.full((out_rows,), -1, np.int64)

    s_off = c_off = r_off = 0
    for K, w, tl, num_idxs, f8 in chunks:
        base = w * WBASE
        nT = 2 * K
        chunk_idx = np.zeros((len(tl), 128), np.int64)
        for jj, (ci, slot_lo, ns) in enumerate(tl):
            _, noff = CFG[ci]
            ts = _tile_bins(ci)
            pc = plans[(ci, w)][core]  # [ni]
            ni = len(pc)
            b_lo = slot_lo // noff
            s = np.arange(128)
            b = b_lo + s // noff
            o = s % noff
            inb = (s < ns) & (b < ni)
            bid = np.where(inb, pc[np.minimum(b, ni - 1)], -1)
            valid = bid >= 0
            ids = np.where(valid, bid, 0)
            iv = (idxP[ids, o].astype(np.int64) - base) * valid
            assert iv.min() >= 0 and iv.max() < WSPAN
            chunk_idx[jj] = iv
            wv = wcP[ids, o, :nT] * valid[:, None]
            wc_arr[:, c_off:c_off + nT] = wv.astype(np.float16)
            rb = b_lo + np.arange(ts)
            rowsel = np.where(rb < ni, pc[np.minimum(rb, ni - 1)], -1)
            slotmap[r_off:r_off + ts] = rowsel
            c_off += nT
            r_off += ts
        # indices i = jj*128 + p -> packed [16, n/16] wrap, replicated x8
        flat = chunk_idx.reshape(-1).astype(np.int16)
        S = len(tl) * 8
        idx_arr[:, s_off:s_off + S] = np.tile(flat.reshape(S, 16).T, (8, 1))
        s_off += S
    return idx_arr, wc_arr, slotmap


def _build_program(chunks, table_dt_name):
    import concourse.bacc as bacc
    import concourse.bass as bass
    import concourse.tile as tile
    import concourse.mybir as mybir
    from contextlib import ExitStack

    tdt = getattr(mybir.dt, table_dt_name)
    Stot = sum(8 * len(tl) for _, _, tl, _, _ in chunks)
    wc_cols = sum(2 * K * len(tl) for K, _, tl, _, _ in chunks)
    out_rows = sum(_tile_bins(ci) for _, _, tl, _, _ in chunks for ci, _, _ in tl)

    nc = bacc.Bacc("TRN2", target_bir_lowering=False, debug=False)
    tbl = nc.dram_tensor("tbl", [TBL_ROWS, 2 * C], tdt, kind="ExternalInput").ap()
    tbl8 = nc.dram_tensor("tbl8", [TBL_ROWS, 2 * C], mybir.dt.float8e4,
                          kind="ExternalInput").ap()
    idxd = nc.dram_tensor("idx", [128, Stot], mybir.dt.int16, kind="ExternalInput").ap()
    wcd = nc.dram_tensor("wc", [128, wc_cols], mybir.dt.float16, kind="ExternalInput").ap()
    maskd = {}
    for noff in (1, 2, 4):
        maskd[noff] = nc.dram_tensor(f"mask{noff}", [128, MAXNT, 128 // noff],
                                     mybir.dt.float16, kind="ExternalInput").ap()
    outd = nc.dram_tensor("out", [out_rows, C], mybir.dt.float16,
                          kind="ExternalOutput").ap()

    with tile.TileContext(nc) as tc, ExitStack() as ctx:
        cpool = ctx.enter_context(tc.tile_pool(name="const", bufs=1))
        gpool = ctx.enter_context(tc.tile_pool(name="g", bufs=4))
        wpool = ctx.enter_context(tc.tile_pool(name="w", bufs=6))
        ppool = ctx.enter_context(tc.tile_pool(name="ps", bufs=8, space="PSUM"))
        opool = ctx.enter_context(tc.tile_pool(name="o", bufs=3))

        idx_sb = cpool.tile([128, Stot], mybir.dt.int16)
        nc.sync.dma_start(idx_sb[:], idxd[:])
        wc_sb = cpool.tile([128, wc_cols], mybir.dt.float16)
        nc.sync.dma_start(wc_sb[:], wcd[:])
        mask_sb = {}
        for noff in (1, 2, 4):
            mask_sb[noff] = cpool.tile([128, MAXNT, 128 // noff], mybir.dt.float16,
                                       name=f"msk{noff}", tag=f"m{noff}")
            nc.sync.dma_start(mask_sb[noff][:, :, :], maskd[noff][:, :, :])

        s_off = c_off = r_off = 0
        for K, w, tl, num_idxs, f8 in chunks:
            m = len(tl)
            nT = 2 * K
            ES = K * 2 * C
            base = w * WBASE
            nwin = min(WSPAN, TBL_ROWS - base - K)
            if f8:
                g = gpool.tile([128, 18432], mybir.dt.float8e4, tag="g8")
                win = bass.AP(tbl8.tensor, base * 2 * C, [[2 * C, nwin], [1, ES]])
            else:
                g = gpool.tile([128, 9216], tdt, tag="g")
                win = bass.AP(tbl.tensor, base * 2 * C, [[2 * C, nwin], [1, ES]])
            nc.gpsimd.dma_gather(
                out_ap=g[:, :m * ES].rearrange("p (m e) -> p m e", e=ES),
                in_ap=win,
                idxs_ap=idx_sb[:, s_off:s_off + num_idxs // 16],
                num_idxs=num_idxs,
                num_idxs_reg=num_idxs,
                elem_size=ES,
                elem_step=2 * C,
            )
            s_off += 8 * m
            crows = sum(_tile_bins(ci) for ci, _, _ in tl)
            ob = opool.tile([128, (crows + 127) // 128, C], mybir.dt.float16,
                            tag="ob")
            row = 0
            for jj, (ci, slot_lo, ns) in enumerate(tl):
                _, noff = CFG[ci]
                ts = _tile_bins(ci)
                msk = mask_sb[noff]
                # the final (trimmed) tile's gather only writes pr partitions;
                # contract only those so the matmul never reads unwritten SBUF
                pr = num_idxs - 128 * jj if jj == m - 1 else 128
                wall = wpool.tile([128, nT, ts], mybir.dt.float16, tag=f"wall{ts}")
                nc.vector.tensor_tensor(
                    out=wall[:, :, :],
                    in0=wc_sb[:, c_off:c_off + nT].to_broadcast([128, nT, ts]),
                    in1=msk[:, :nT, :],
                    op=mybir.AluOpType.mult,
                )
                psum = ppool.tile([ts, C], mybir.dt.float32, tag="ps")
                for j in range(nT):
                    nc.tensor.matmul(
                        psum[:],
                        lhsT=wall[:pr, j, :],
                        rhs=g[:pr, jj * ES + j * C:jj * ES + (j + 1) * C],
                        start=(j == 0),
                        stop=(j == nT - 1),
                    )
                # partition-group rule: an AP starting at partition s may span
                # at most 128/64/32 partitions for s = 0/64/{32,96}
                done = 0
                while done < ts:
                    r = (row + done) % 128
                    allow = (lambda s: 128 if s == 0 else 64 if s == 64 else 32)
                    seg = min(ts - done, allow(r), allow(done), 128 - r)
                    nc.scalar.copy(ob[r:r + seg, (row + done) // 128, :],
                                   psum[done:done + seg, :])
                    done += seg
                c_off += nT
                row += ts
            qf = crows // 128
            if qf:
                nc.sync.dma_start(
                    outd[r_off:r_off + qf * 128, :].rearrange(
                        "(q p) c -> p q c", p=128),
                    ob[:, :qf, :])
            if crows % 128:
                nc.sync.dma_start(
                    outd[r_off + qf * 128:r_off + crows, :],
                    ob[:crows % 128, qf, :])
            r_off += crows

    nc.compile()
    return nc


def _make_mask():
    masks = {}
    p = np.arange(128)
    for noff in (1, 2, 4):
        m = np.zeros((128, MAXNT, 128 // noff), np.float16)
        for j in range(MAXNT):
            m[p, j, p // noff] = 1.0
        masks[noff] = m
    return masks


def prepare(feat0, feat1, feat2, feat3, boxes, img_ids, table_dt="float16"):
    import ml_dtypes

    np_dt = np.float16 if table_dt == "float16" else np.float32
    tbl = _make_table((feat0, feat1, feat2, feat3), np_dt)
    tbl8 = tbl.astype(ml_dtypes.float8_e4m3fn)
    cls, idxP, wcP = _host_precompute(boxes, img_ids)
    plans, chunks = _plan(cls, idxP)

    sig = (_sig(chunks), table_dt)
    if sig not in _PROGRAM_CACHE:
        _PROGRAM_CACHE[sig] = _build_program(chunks, table_dt)
    nc = _PROGRAM_CACHE[sig]

    masks = _make_mask()
    in_maps = []
    slotmaps = []
    for c in range(NCORES):
        idx_arr, wc_arr, slotmap = _pack_core(c, plans, chunks, idxP, wcP)
        im = {"tbl": tbl, "tbl8": tbl8, "idx": idx_arr, "wc": wc_arr}
        for noff in (1, 2, 4):
            im[f"mask{noff}"] = masks[noff]
        in_maps.append(im)
        slotmaps.append(slotmap)
    return nc, in_maps, slotmaps


def assemble(results, slotmaps):
    final = np.zeros((N, C, NBIN), np.float32)
    for c in range(NCORES):
        out = results[c]["out"]
        sm = slotmaps[c]
        valid = sm >= 0
        ids = sm[valid]
        final[ids // NBIN, :, ids % NBIN] = out[valid].astype(np.float32)
    return final.reshape(N, C, OUT, OUT)


def kernel(feat0, feat1, feat2, feat3, boxes, img_ids):
    from concourse.bass_utils import run_bass_kernel_spmd

    nc, in_maps, slotmaps = prepare(feat0, feat1, feat2, feat3, boxes, img_ids)
    res = run_bass_kernel_spmd(nc, in_maps, list(range(NCORES)))
    return assemble(res.results, slotmaps)
